# revision 27
# baseline (speedup 1.0000x reference)
"""CCPL contrastive-loss kernel for Trainium2 (8 NeuronCores).

Strategy: the loss only touches 256 sampled 3x3 neighborhoods of
feat_q/feat_k, so the kernel never streams the full tensors.  Work is
data-parallel over the batch dim: core b gets batch b's q and k stacked
channels-LAST as one [H*W, 2C] array in HBM.  In that layout a window
row (3 cols x 128 channels) is one contiguous 1536B run, and landing
each sample on its own SBUF partition makes it a single DMA descriptor.
The whole gather is TWO gpsimd indirect DMAs (one per 128-sample
layer, pipelined against compute) driven by a [128, 6] int32 row-index
tensor, instead of hundreds of strided dma_starts paying the ~630ns
fixed HWDGE cost each.

Compute is fully 128-lane and mostly bf16 (the tensor is staged bf16,
so subtract and q-k difference hit the DVE 16-bit 2x path): samples
live on partitions, (position, channel) on the free dim.  d = window -
center; norm2 via ACT square + DVE grouped reduce over the 64-channel
runs; nrm = sqrt(n2 + 1e-14) (== sqrt(n2) + 1e-7 at n2 == 0, matching
the reference eps exactly where it matters); rinv = 1/nrm; qhat = d *
rinv; |qhat - khat| is abs-summed per partition by the ACT Abs
activation's accumulator, folded across partitions by one PE matmul,
and the host sums the 8 per-core scalars.
"""

import os
import sys
from contextlib import ExitStack

import numpy as np

sys.path.insert(0, "/opt/trn_rl_repo")

import concourse.bass as bass
import concourse.tile as tile
from concourse import mybir
from concourse.bass_utils import run_bass_kernel_spmd


def _install_ntff_hook():
    """Provide antenv.axon_hooks when the agent image lacks it.

    concourse's axon trace path imports antenv.axon_hooks to fetch the
    NTFF profile hook; this image's antenv has no such submodule.  The
    hook implementation ships in trn_agent_boot.trn_boot, so wire it up
    against the axon PJRT .so directly.
    """
    try:
        from antenv.axon_hooks import get_axon_ntff_profile_hook  # noqa: F401

        return
    except ImportError:
        pass
    import types

    hook = None
    try:
        from trn_agent_boot.trn_boot import _ntff_profile_via_ctypes

        so = "/opt/axon/libaxon_pjrt.so"
        if os.path.exists(so):
            hook = _ntff_profile_via_ctypes(so)
    except Exception:
        hook = None
    mod = types.ModuleType("antenv.axon_hooks")
    _state = {"hook": hook}
    mod.get_axon_ntff_profile_hook = lambda: _state["hook"]
    mod.set_axon_ntff_profile_hook = lambda h: _state.update(hook=h)
    import antenv

    sys.modules["antenv.axon_hooks"] = mod
    antenv.axon_hooks = mod


_install_ntff_hook()

B, C, H, W = 8, 64, 512, 512
NUM_S = 256
DELTA = 1e-14  # sqrt(n2 + DELTA): equals sqrt(n2)+1e-7 at n2==0 (center cols)
P = 128  # samples per partition-layer; 2C stacked channels
NSHI = NUM_S // P  # 2 sample layers per partition
NROW = 3 * NSHI  # window rows gathered per partition
N_CORES = 8

_nc_cache = None
LAST_RESULTS = None  # BassKernelResults of the most recent run (for test.py)


def _split_multi_waits(nc):
    """Walrus build here embeds at most ONE sync wait per instruction.

    Tile emits instructions (notably the kernel-tail Drain) carrying many
    sem waits.  Hoist all but the last wait of any such instruction onto
    single-wait NOPs inserted immediately before it on the same queue —
    the queue stalls on each NOP in turn, preserving semantics.
    """
    from concourse import mybir as _mybir

    for f in nc.m.functions:
        for blk in f.blocks:
            insts = blk.instructions
            i = 0
            while i < len(insts):
                inst = insts[i]
                si = inst.sync_info
                if si is not None and si.on_wait and len(si.on_wait) > 1:
                    waits = list(si.on_wait)
                    si.on_wait = waits[-1:]
                    for j, w in enumerate(waits[:-1]):
                        nop = _mybir.InstNoOp(
                            name=nc.get_next_instruction_name(),
                            ins=[],
                            outs=[],
                            engine=inst.engine,
                            sync_info=_mybir.SyncInfo(on_wait=[w], on_update=[]),
                        )
                        insts.insert(i + j, nop)
                    i += len(waits) - 1
                i += 1


def _hoist_idx_dma(nc):
    """Issue the idx staging DMA before the block-0 all-engine barrier.

    The preamble barrier gates block 1 on Pool's const-AP memsets (~1.5us
    after the SP queue is ready).  The idx DMA has no waits and touches
    only its own tile, so moving it onto the SP stream ahead of the
    barrier starts the gather's 2.4us HBM round trip that much earlier.
    """
    from concourse import mybir as _mybir

    blocks = nc.m.functions[0].blocks
    if len(blocks) < 2:
        return
    b0, b1 = blocks[0], blocks[1]
    dma = None
    for inst in b1.instructions:
        if (
            isinstance(inst, _mybir.InstDMACopy)
            and inst.engine == _mybir.EngineType.SP
            and not (inst.sync_info and inst.sync_info.on_wait)
        ):
            dma = inst
            break
    if dma is None:
        return
    b1.instructions.remove(dma)
    for i, inst in enumerate(b0.instructions):
        if inst.engine == _mybir.EngineType.SP:
            b0.instructions.insert(i, dma)
            return
    b0.instructions.append(dma)


def _build():
    f32 = mybir.dt.float32
    bf16 = mybir.dt.bfloat16
    i32 = mybir.dt.int32
    nc = bass.Bass()
    # channels-last: flat (h*W + w) rows of 2C channels (q in 0..C, k in C..2C)
    fqkT = nc.dram_tensor("fqkT", [H * W, 2 * C], bf16, kind="ExternalInput")
    # per (partition, shi*3 + r): DRAM row index (h+r)*W + w of sample shi*128+p
    idx = nc.dram_tensor("idx", [P, NROW], i32, kind="ExternalInput")
    out = nc.dram_tensor("out", [P, NSHI], f32, kind="ExternalOutput")

    with tile.TileContext(nc) as tc, ExitStack() as ctx:
        sb = ctx.enter_context(tc.tile_pool(name="sb", bufs=1))

        idx_sb = sb.tile([P, NROW], i32)
        nc.sync.dma_start(out=idx_sb[:], in_=idx[:])
        deltas = sb.tile([P, 1], f32)
        nc.vector.memset(deltas[:], DELTA)

        # Per-phase tiles (phase = one 128-sample layer).  fqkT is staged
        # bf16 from the host, halving gather bytes; subtract and q-k diff
        # hit the DVE 16-bit 2x path, reductions accumulate in f32.
        qk = [sb.tile([P, 9, 2 * C], bf16, name=f"qk{s}") for s in range(NSHI)]
        d = [sb.tile([P, 9, 2 * C], bf16, name=f"d{s}") for s in range(NSHI)]
        d2 = [sb.tile([P, 9, 2 * C], bf16, name=f"d2{s}") for s in range(NSHI)]
        n2 = [sb.tile([P, 9, 2, 1], bf16, name=f"n2{s}") for s in range(NSHI)]
        rinv = [sb.tile([P, 9, 2, 1], bf16, name=f"ri{s}") for s in range(NSHI)]
        dif = [sb.tile([P, 9, C], bf16, name=f"df{s}") for s in range(NSHI)]
        acc = sb.tile([P, NSHI], f32)

        # Gathers first: one indirect DMA per phase; per index, 384
        # consecutive floats from fqkT (= 3 w-positions x 128 channels)
        # land contiguously in one partition.
        for s in range(NSHI):
            nc.gpsimd.indirect_dma_start(
                out=qk[s][:].rearrange("p n c -> p (n c)"),
                out_offset=None,
                in_=fqkT[:],
                in_offset=bass.IndirectOffsetOnAxis(
                    ap=idx_sb[:, 3 * s : 3 * s + 3], axis=0
                ),
            )

        def sub(s):  # d = window - center (center column pos=4 becomes 0)
            nc.vector.tensor_tensor(
                out=d[s][:],
                in0=qk[s][:],
                in1=qk[s][:, 4:5, :].to_broadcast([P, 9, 2 * C]),
                op=mybir.AluOpType.subtract,
            )

        def square(s):
            nc.scalar.square(out=d2[s][:], in_=d[s][:])

        def red(s):  # norm2 over each 64-channel run, q and k separately
            with nc.allow_low_precision(reason="bf16 norm2, 0.2% on norms"):
                nc.vector.tensor_reduce(
                    out=n2[s][:],
                    in_=d2[s][:].rearrange("p n (t c) -> p n t c", t=2),
                    axis=mybir.AxisListType.X,
                    op=mybir.AluOpType.add,
                )

        def vsqrt(s):  # nrm = sqrt(n2 + delta)
            nc.scalar.activation(
                out=n2[s][:],
                in_=n2[s][:],
                func=mybir.ActivationFunctionType.Sqrt,
                bias=deltas[:],
            )

        def recip(s):
            # bf16 rinv keeps the downstream mult on the pure-bf16 2x path;
            # the ~0.4% per-column scale noise averages out in the loss.
            with nc.allow_low_precision(reason="bf16 rinv, error averages out"):
                nc.vector.reciprocal(out=rinv[s][:], in_=n2[s][:])

        def mult(s):  # qhat/khat = d * rinv; center cols give 0 * (1/1e-7) = 0
            nc.vector.tensor_tensor(
                out=d2[s][:].rearrange("p n (t c) -> p n t c", t=2),
                in0=d[s][:].rearrange("p n (t c) -> p n t c", t=2),
                in1=rinv[s][:].to_broadcast([P, 9, 2, C]),
                op=mybir.AluOpType.mult,
            )

        def qkdif(s):
            qhv = d2[s][:].rearrange("p n (t c) -> p n t c", t=2)
            nc.vector.tensor_tensor(
                out=dif[s][:],
                in0=qhv[:, :, 0, :],
                in1=qhv[:, :, 1, :],
                op=mybir.AluOpType.subtract,
            )

        def absred(s):  # on ACT: |dif| with running-sum accumulator
            nc.scalar.activation(
                out=dif[s][:],
                in_=dif[s][:],
                func=mybir.ActivationFunctionType.Abs,
                accum_out=acc[:, s : s + 1],
            )

        # Two-phase software pipeline; emission order fixes per-engine
        # queue order, Tile inserts the cross-engine semaphores.
        sub(0)
        square(0)
        sub(1)
        red(0)
        square(1)
        vsqrt(0)
        recip(0)
        mult(0)
        qkdif(0)
        red(1)
        vsqrt(1)
        absred(0)
        recip(1)
        mult(1)
        qkdif(1)
        absred(1)
        nc.sync.dma_start(out=out[:], in_=acc[:])

    _split_multi_waits(nc)
    _hoist_idx_dma(nc)
    return nc


def kernel(feat_q, feat_k, sample_ids, *, trace=False, trace_cores=None):
    global LAST_RESULTS, _nc_cache
    feat_q = np.asarray(feat_q, dtype=np.float32)
    feat_k = np.asarray(feat_k, dtype=np.float32)
    ids = np.asarray(sample_ids).astype(np.int64)
    if _nc_cache is None:
        _nc_cache = _build()
    nc = _nc_cache

    # idx[p, shi*3 + r] = (h + r)*W + w for sample s = shi*128 + p
    hw = ids[:, 0] * W + ids[:, 1]  # [256]
    rows = hw[:, None] + np.arange(3, dtype=np.int64)[None, :] * W  # [256, 3]
    idx_np = np.ascontiguousarray(
        rows.reshape(NSHI, P, 3).transpose(1, 0, 2).reshape(P, NROW)
    ).astype(np.int32)

    import ml_dtypes

    in_maps = []
    for b in range(N_CORES):
        x = np.concatenate([feat_q[b], feat_k[b]], axis=0)  # [128, H, W]
        fqkT = np.ascontiguousarray(
            x.transpose(1, 2, 0).astype(ml_dtypes.bfloat16)
        ).reshape(H * W, 2 * C)
        in_maps.append({"fqkT": fqkT, "idx": idx_np})

    results = run_bass_kernel_spmd(
        nc,
        in_maps,
        core_ids=list(range(N_CORES)),
        trace=trace,
        trace_cores=trace_cores,
    )
    LAST_RESULTS = results
    total = np.float64(0.0)
    for r in results.results:
        total += np.float64(np.sum(np.asarray(r["out"], dtype=np.float64)))
    loss = total / (B * C * 8 * NUM_S)
    return np.asarray(loss, dtype=np.float32)


if __name__ == "__main__":
    # quick smoke test against random data
    rng = np.random.default_rng(0)
    fq = rng.standard_normal((B, C, H, W), dtype=np.float32)
    fk = rng.standard_normal((B, C, H, W), dtype=np.float32)
    ids = rng.integers(0, H - 2, size=(NUM_S, 2))
    print(kernel(fq, fk, ids))


# revision 28
# speedup vs baseline: 1.2423x; 1.2423x over previous
"""CCPL contrastive-loss kernel for Trainium2 (8 NeuronCores).

Strategy: the loss only touches 256 sampled 3x3 neighborhoods of
feat_q/feat_k, so the kernel never streams the full tensors.  Work is
data-parallel over the batch dim: core b gets batch b's q and k stacked
channels-LAST as one [H*W, 2C] array in HBM.  In that layout a window
row (3 cols x 128 channels) is one contiguous 1536B run, and landing
each sample on its own SBUF partition makes it a single DMA descriptor.
The whole gather is TWO gpsimd indirect DMAs (one per 128-sample
layer, pipelined against compute) driven by a [128, 6] int32 row-index
tensor, instead of hundreds of strided dma_starts paying the ~630ns
fixed HWDGE cost each.

Compute is fully 128-lane and mostly bf16 (the tensor is staged bf16,
so subtract and q-k difference hit the DVE 16-bit 2x path): samples
live on partitions, (position, channel) on the free dim.  d = window -
center; norm2 via ACT square + DVE grouped reduce over the 64-channel
runs; nrm = sqrt(n2 + 1e-14) (== sqrt(n2) + 1e-7 at n2 == 0, matching
the reference eps exactly where it matters); rinv = 1/nrm; qhat = d *
rinv; |qhat - khat| is abs-summed per partition by the ACT Abs
activation's accumulator, folded across partitions by one PE matmul,
and the host sums the 8 per-core scalars.
"""

import os
import sys
from contextlib import ExitStack

import numpy as np

sys.path.insert(0, "/opt/trn_rl_repo")

import concourse.bass as bass
import concourse.tile as tile
from concourse import mybir
from concourse.bass_utils import run_bass_kernel_spmd


def _install_ntff_hook():
    """Provide antenv.axon_hooks when the agent image lacks it.

    concourse's axon trace path imports antenv.axon_hooks to fetch the
    NTFF profile hook; this image's antenv has no such submodule.  The
    hook implementation ships in trn_agent_boot.trn_boot, so wire it up
    against the axon PJRT .so directly.
    """
    try:
        from antenv.axon_hooks import get_axon_ntff_profile_hook  # noqa: F401

        return
    except ImportError:
        pass
    import types

    hook = None
    try:
        from trn_agent_boot.trn_boot import _ntff_profile_via_ctypes

        so = "/opt/axon/libaxon_pjrt.so"
        if os.path.exists(so):
            hook = _ntff_profile_via_ctypes(so)
    except Exception:
        hook = None
    mod = types.ModuleType("antenv.axon_hooks")
    _state = {"hook": hook}
    mod.get_axon_ntff_profile_hook = lambda: _state["hook"]
    mod.set_axon_ntff_profile_hook = lambda h: _state.update(hook=h)
    import antenv

    sys.modules["antenv.axon_hooks"] = mod
    antenv.axon_hooks = mod


_install_ntff_hook()

B, C, H, W = 8, 64, 512, 512
NUM_S = 256
DELTA = 1e-14  # sqrt(n2 + DELTA): equals sqrt(n2)+1e-7 at n2==0 (center cols)
P = 128  # samples per partition-layer; 2C stacked channels
NSHI = NUM_S // P  # 2 sample layers per partition
NROW = 3 * NSHI  # window rows gathered per partition
N_CORES = 8

_nc_cache = None
LAST_RESULTS = None  # BassKernelResults of the most recent run (for test.py)


def _split_multi_waits(nc):
    """Walrus build here embeds at most ONE sync wait per instruction.

    Tile emits instructions (notably the kernel-tail Drain) carrying many
    sem waits.  Hoist all but the last wait of any such instruction onto
    single-wait NOPs inserted immediately before it on the same queue —
    the queue stalls on each NOP in turn, preserving semantics.
    """
    from concourse import mybir as _mybir

    for f in nc.m.functions:
        for blk in f.blocks:
            insts = blk.instructions
            i = 0
            while i < len(insts):
                inst = insts[i]
                si = inst.sync_info
                if si is not None and si.on_wait and len(si.on_wait) > 1:
                    waits = list(si.on_wait)
                    si.on_wait = waits[-1:]
                    for j, w in enumerate(waits[:-1]):
                        nop = _mybir.InstNoOp(
                            name=nc.get_next_instruction_name(),
                            ins=[],
                            outs=[],
                            engine=inst.engine,
                            sync_info=_mybir.SyncInfo(on_wait=[w], on_update=[]),
                        )
                        insts.insert(i + j, nop)
                    i += len(waits) - 1
                i += 1


def _hoist_idx_dma(nc):
    """Issue the idx staging DMA before the block-0 all-engine barrier.

    The preamble barrier gates block 1 on Pool's const-AP memsets (~1.5us
    after the SP queue is ready).  The idx DMA has no waits and touches
    only its own tile, so moving it onto the SP stream ahead of the
    barrier starts the gather's 2.4us HBM round trip that much earlier.
    """
    from concourse import mybir as _mybir

    blocks = nc.m.functions[0].blocks
    if len(blocks) < 2:
        return
    b0, b1 = blocks[0], blocks[1]
    dma = None
    for inst in b1.instructions:
        if (
            isinstance(inst, _mybir.InstDMACopy)
            and inst.engine == _mybir.EngineType.SP
            and not (inst.sync_info and inst.sync_info.on_wait)
        ):
            dma = inst
            break
    if dma is None:
        return
    b1.instructions.remove(dma)
    for i, inst in enumerate(b0.instructions):
        if inst.engine == _mybir.EngineType.SP:
            b0.instructions.insert(i, dma)
            return
    b0.instructions.append(dma)


def _build():
    f32 = mybir.dt.float32
    bf16 = mybir.dt.bfloat16
    i32 = mybir.dt.int32
    nc = bass.Bass()
    # channels-last: flat (h*W + w) rows of 2C channels (q in 0..C, k in C..2C)
    fqkT = nc.dram_tensor("fqkT", [H * W, 2 * C], bf16, kind="ExternalInput")
    # per (partition, shi*3 + r): DRAM row index (h+r)*W + w of sample shi*128+p
    idx = nc.dram_tensor("idx", [P, NROW], i32, kind="ExternalInput")
    out = nc.dram_tensor("out", [1, NSHI], f32, kind="ExternalOutput")

    with tile.TileContext(nc) as tc, ExitStack() as ctx:
        sb = ctx.enter_context(tc.tile_pool(name="sb", bufs=1))
        pf = ctx.enter_context(tc.tile_pool(name="pf", bufs=1, space="PSUM"))

        idx_sb = sb.tile([P, NROW], i32)
        nc.sync.dma_start(out=idx_sb[:], in_=idx[:])
        deltas = sb.tile([P, 1], f32)
        nc.vector.memset(deltas[:], DELTA)
        ones = sb.tile([P, 1], f32)
        nc.vector.memset(ones[:], 1.0)
        # PE warmup so the final matmul doesn't pay a fresh clock wait.
        warm = pf.tile([1, 1], f32, tag="warm")
        nc.tensor.matmul(
            out=warm[:], lhsT=ones[:], rhs=ones[:], start=True, stop=True
        )

        # Per-phase tiles (phase = one 128-sample layer).  fqkT is staged
        # bf16 from the host, halving gather bytes; subtract and q-k diff
        # hit the DVE 16-bit 2x path, reductions accumulate in f32.
        qk = [sb.tile([P, 9, 2 * C], bf16, name=f"qk{s}") for s in range(NSHI)]
        d = [sb.tile([P, 9, 2 * C], bf16, name=f"d{s}") for s in range(NSHI)]
        d2 = [sb.tile([P, 9, 2 * C], bf16, name=f"d2{s}") for s in range(NSHI)]
        n2 = [sb.tile([P, 9, 2, 1], bf16, name=f"n2{s}") for s in range(NSHI)]
        rinv = [sb.tile([P, 9, 2, 1], bf16, name=f"ri{s}") for s in range(NSHI)]
        dif = [sb.tile([P, 9, C], bf16, name=f"df{s}") for s in range(NSHI)]
        acc = sb.tile([P, NSHI], f32)

        # Gathers first: one indirect DMA per phase; per index, 384
        # consecutive floats from fqkT (= 3 w-positions x 128 channels)
        # land contiguously in one partition.
        for s in range(NSHI):
            nc.gpsimd.indirect_dma_start(
                out=qk[s][:].rearrange("p n c -> p (n c)"),
                out_offset=None,
                in_=fqkT[:],
                in_offset=bass.IndirectOffsetOnAxis(
                    ap=idx_sb[:, 3 * s : 3 * s + 3], axis=0
                ),
            )

        def sub(s):  # d = window - center (center column pos=4 becomes 0)
            nc.vector.tensor_tensor(
                out=d[s][:],
                in0=qk[s][:],
                in1=qk[s][:, 4:5, :].to_broadcast([P, 9, 2 * C]),
                op=mybir.AluOpType.subtract,
            )

        def square(s):
            nc.scalar.square(out=d2[s][:], in_=d[s][:])

        def red(s):  # norm2 over each 64-channel run, q and k separately
            with nc.allow_low_precision(reason="bf16 norm2, 0.2% on norms"):
                nc.vector.tensor_reduce(
                    out=n2[s][:],
                    in_=d2[s][:].rearrange("p n (t c) -> p n t c", t=2),
                    axis=mybir.AxisListType.X,
                    op=mybir.AluOpType.add,
                )

        def vsqrt(s):  # nrm = sqrt(n2 + delta)
            nc.scalar.activation(
                out=n2[s][:],
                in_=n2[s][:],
                func=mybir.ActivationFunctionType.Sqrt,
                bias=deltas[:],
            )

        def recip(s):
            # bf16 rinv keeps the downstream mult on the pure-bf16 2x path;
            # the ~0.4% per-column scale noise averages out in the loss.
            with nc.allow_low_precision(reason="bf16 rinv, error averages out"):
                nc.vector.reciprocal(out=rinv[s][:], in_=n2[s][:])

        def mult(s):  # qhat/khat = d * rinv; center cols give 0 * (1/1e-7) = 0
            nc.vector.tensor_tensor(
                out=d2[s][:].rearrange("p n (t c) -> p n t c", t=2),
                in0=d[s][:].rearrange("p n (t c) -> p n t c", t=2),
                in1=rinv[s][:].to_broadcast([P, 9, 2, C]),
                op=mybir.AluOpType.mult,
            )

        def qkdif(s):
            qhv = d2[s][:].rearrange("p n (t c) -> p n t c", t=2)
            nc.vector.tensor_tensor(
                out=dif[s][:],
                in0=qhv[:, :, 0, :],
                in1=qhv[:, :, 1, :],
                op=mybir.AluOpType.subtract,
            )

        def absred(s):  # on ACT: |dif| with running-sum accumulator
            nc.scalar.activation(
                out=dif[s][:],
                in_=dif[s][:],
                func=mybir.ActivationFunctionType.Abs,
                accum_out=acc[:, s : s + 1],
            )

        # Two-phase software pipeline; emission order fixes per-engine
        # queue order, Tile inserts the cross-engine semaphores.
        sub(0)
        square(0)
        sub(1)
        red(0)
        square(1)
        vsqrt(0)
        recip(0)
        mult(0)
        qkdif(0)
        red(1)
        vsqrt(1)
        absred(0)
        recip(1)
        mult(1)
        qkdif(1)
        absred(1)
        # fold the 128 partitions on PE: out[1, s] = sum_p acc[p, s].
        # One matmul per phase column so phase 0's fold overlaps phase 1.
        pfin = pf.tile([1, NSHI], f32, tag="fin")
        for s in range(NSHI):
            nc.tensor.matmul(
                out=pfin[:, s : s + 1],
                lhsT=ones[:],
                rhs=acc[:, s : s + 1],
                start=True,
                stop=True,
            )
        res = sb.tile([1, NSHI], f32)
        nc.scalar.copy(out=res[:], in_=pfin[:])
        nc.sync.dma_start(out=out[:], in_=res[:])

    _split_multi_waits(nc)
    _hoist_idx_dma(nc)
    return nc


def kernel(feat_q, feat_k, sample_ids, *, trace=False, trace_cores=None):
    global LAST_RESULTS, _nc_cache
    feat_q = np.asarray(feat_q, dtype=np.float32)
    feat_k = np.asarray(feat_k, dtype=np.float32)
    ids = np.asarray(sample_ids).astype(np.int64)
    if _nc_cache is None:
        _nc_cache = _build()
    nc = _nc_cache

    # idx[p, shi*3 + r] = (h + r)*W + w for sample s = shi*128 + p
    hw = ids[:, 0] * W + ids[:, 1]  # [256]
    rows = hw[:, None] + np.arange(3, dtype=np.int64)[None, :] * W  # [256, 3]
    idx_np = np.ascontiguousarray(
        rows.reshape(NSHI, P, 3).transpose(1, 0, 2).reshape(P, NROW)
    ).astype(np.int32)

    import ml_dtypes

    in_maps = []
    for b in range(N_CORES):
        x = np.concatenate([feat_q[b], feat_k[b]], axis=0)  # [128, H, W]
        fqkT = np.ascontiguousarray(
            x.transpose(1, 2, 0).astype(ml_dtypes.bfloat16)
        ).reshape(H * W, 2 * C)
        in_maps.append({"fqkT": fqkT, "idx": idx_np})

    results = run_bass_kernel_spmd(
        nc,
        in_maps,
        core_ids=list(range(N_CORES)),
        trace=trace,
        trace_cores=trace_cores,
    )
    LAST_RESULTS = results
    total = np.float64(0.0)
    for r in results.results:
        total += np.float64(np.sum(np.asarray(r["out"], dtype=np.float64)))
    loss = total / (B * C * 8 * NUM_S)
    return np.asarray(loss, dtype=np.float32)


if __name__ == "__main__":
    # quick smoke test against random data
    rng = np.random.default_rng(0)
    fq = rng.standard_normal((B, C, H, W), dtype=np.float32)
    fk = rng.standard_normal((B, C, H, W), dtype=np.float32)
    ids = rng.integers(0, H - 2, size=(NUM_S, 2))
    print(kernel(fq, fk, ids))
